# revision 11
# baseline (speedup 1.0000x reference)
"""Two-layer GAT (PyG GATConv semantics) on 8 TRN2 NeuronCores.

v2 strategy (edge/graph parallel, dst-sharded, dma_gather based):
  - Host (index manipulation only): sort edges by dst, shard dst nodes
    contiguously across 8 cores.  Per core, per dst tile (128 nodes), split
    edges into 4 src-range buckets (so gather indices fit int16) and pad
    each (tile, bucket) chunk to a multiple of 128 edges.  The column
    schedule (tile/bucket layout) is shared across cores (max over cores);
    per-core arrays carry the actual indices (int16, wrapped [16, n/16]
    replicated over the 8 partition groups) and dst-local ids (bf16, -1
    for padding).
  - Phase A (per core, redundant): T1[v] = [h | a_src | a_dst | pad] with
    h = x@W1 (bias re-added after softmax normalization in the flush),
    a_src/a_dst folded into the GEMM rhs.  Rows are 384 cols bf16 = 768B
    (dma_gather requires 256B multiples).  Rotated per core so the own
    dst shard sits at rows [0, SH).  Also writes ADT [SH, 128]: compact
    a_dst rows for the own shard.
  - Phase B (layer-1 edge phase): per (group of G dst tiles, bucket):
    one dma_gather for src rows, one for per-edge a_dst (256B rows from
    ADT, indexed by dst-local id).  Incidence matrices S^T (edge x dst)
    are built on device with one broadcast is_equal against an iota
    constant.  alpha = exp(leakyrelu(a_src[src]+a_dst[dst])); one
    accumulating matmul per 128-edge column aggregates [alpha*h | alpha]
    into the dst tile's PSUM.  Self loops are added densely in the
    flush; softmax denominator folded in; +bias; ELU -> Z.
  - Phase C: T2[v] = [z@W2 | a2_src | a2_dst | pad] (128 cols bf16 =
    256B rows) per-core shard; one AllGather -> T2F.
  - Phase D: layer-2 edge phase (heads=1), same structure; a2_dst from
    T2L (local shard) gathers; + bias2 -> out.

All floating-point math happens on device; the host only reorders
indices, pads, and casts layouts.
"""

import contextlib

import numpy as np

import concourse.bass as bass
import concourse.bacc as bacc
import concourse.mybir as mybir
import concourse.tile as tile
from concourse.bass_utils import run_bass_kernel_spmd

# ---- fixed problem hyperparameters (from the nn.Module) ----
F_IN = 256
H = 8
C = 32
NCLS = 40
NEG = 0.2

W = 8               # cores
P = 128             # partitions
NBK = 4             # src-range buckets (int16 gather indices)
G = 6               # dst tiles per group (PSUM banks for agg)
CMAX = 44           # max columns per gather chunk (SBUF)
ES1 = 384           # layer-1 table row (bf16 cols; 768B)
D1 = F_IN + 2 * H   # useful layer-1 row: [h 256 | asrc 8 | adst 8]
ES2 = 128           # layer-2 table row (bf16 cols; 256B)
D2 = NCLS + 2       # useful layer-2 row: [h2 40 | a2s 1 | a2d 1]

f32 = mybir.dt.float32
bf16 = mybir.dt.bfloat16
i16 = mybir.dt.int16
BF_NP = mybir.dt.np(bf16)

Exp = mybir.ActivationFunctionType.Exp
Copy = mybir.ActivationFunctionType.Copy
ADD = mybir.AluOpType.add
MULT = mybir.AluOpType.mult
MAX = mybir.AluOpType.max
ISEQ = mybir.AluOpType.is_equal

TRACE = False       # set by test harness for profiling runs
_CACHE = {}


# --------------------------------------------------------------------------
# host-side index preprocessing
# --------------------------------------------------------------------------

def _wrap16(ix, n_pad):
    """int array -> [128, n_pad/16] i16: idx i at [i%16, i//16], replicated
    across the 8 groups of 16 partitions (one per Q7 core)."""
    full = np.zeros(n_pad, np.int16)
    full[:len(ix)] = ix.astype(np.int16)
    base = full.reshape(n_pad // 16, 16).T          # [16, n/16]
    return np.tile(base, (8, 1))                    # [128, n/16]


def _schedule(counts, Tn):
    """counts: [W, T, NBK].  Shared schedule: per-(tile,bucket) column
    counts (max over cores), chunked into gather ops of <= CMAX columns,
    grouped G dst tiles at a time."""
    K = -(-counts.max(axis=0) // P)                 # [T, NBK]
    n_groups = -(-Tn // G)
    ops = []                                        # (g, b, [(t, k), ...])
    for g in range(n_groups):
        tiles = list(range(g * G, min((g + 1) * G, Tn)))
        for b in range(NBK):
            cols = []
            for t in tiles:
                for k in range(int(K[t, b])):
                    cols.append((t, k))
            for c0 in range(0, len(cols), CMAX):
                ops.append((g, b, cols[c0:c0 + CMAX]))
    # column index of each (t, b, k) in schedule order; per-op col offset.
    # (t, b)'s K columns are consecutive in schedule order, so a base-offset
    # array suffices.
    colbase = np.full((Tn, NBK), -1, np.int64)
    coff = []
    cur = 0
    for (g, b, cols) in ops:
        coff.append(cur)
        for (t, k) in cols:
            if k == 0:
                colbase[t, b] = cur
            cur += 1
    CT = cur
    return K, ops, colbase, coff, CT


def _layer_arrays(loc, bkt, t_all, dstl, dstloc, K, ops, colbase, CT):
    """Per-core packed arrays for one layer's schedule."""
    Tn = K.shape[0]
    order = np.lexsort((bkt, t_all))
    loc_s, b_s, t_s = loc[order], bkt[order], t_all[order]
    dstl_s, dstloc_s = dstl[order], dstloc[order]

    cnt = np.bincount(t_s * NBK + b_s, minlength=Tn * NBK)
    starts = np.zeros(Tn * NBK + 1, np.int64)
    starts[1:] = np.cumsum(cnt)

    pos = np.arange(len(t_s)) - starts[t_s * NBK + b_s]
    k_e = pos // P
    p_e = pos % P
    col_e = colbase[t_s, b_s] + k_e

    dstl_cols = np.full((P, max(1, CT)), -1.0, np.float32)
    dstl_cols[p_e, col_e] = dstl_s
    loc_full = np.zeros((P, max(1, CT)), np.int64)
    dloc_full = np.zeros((P, max(1, CT)), np.int64)
    loc_full[p_e, col_e] = loc_s
    dloc_full[p_e, col_e] = dstloc_s

    iw_parts, iwd_parts = [], []
    cur = 0
    for (g, b, cols) in ops:
        ncol = len(cols)
        ci = list(range(cur, cur + ncol))
        cur += ncol
        ni = ncol * P
        lin_src = loc_full[:, ci].T.reshape(-1)     # column-major, p fastest
        lin_dst = dloc_full[:, ci].T.reshape(-1)
        iw_parts.append(_wrap16(lin_src, ni))
        iwd_parts.append(_wrap16(lin_dst, ni))
    iw = (np.concatenate(iw_parts, axis=1) if iw_parts
          else np.zeros((P, 8), np.int16))
    iwd = (np.concatenate(iwd_parts, axis=1) if iwd_parts
           else np.zeros((P, 8), np.int16))
    return iw, iwd, dstl_cols.astype(BF_NP)


def _host_prep(x, edge_index):
    N = x.shape[0]
    E = edge_index.shape[1]
    src_a = np.asarray(edge_index[0], np.int64)
    dst_a = np.asarray(edge_index[1], np.int64)

    nt_real = -(-N // P)
    T = -(-nt_real // W)
    SH = T * P
    NPAD = W * SH
    BUCK = NPAD // NBK

    counts1 = np.zeros((W, T, NBK), np.int64)
    counts2 = np.zeros((W, T, NBK), np.int64)
    pc = []
    for c in range(W):
        m = (dst_a >= c * SH) & (dst_a < (c + 1) * SH)
        src, dst = src_a[m], dst_a[m]
        t_all = (dst - c * SH) // P
        dstl = (dst - c * SH) % P
        dstloc = dst - c * SH
        src_rot = (src - c * SH) % NPAD
        b1, l1 = src_rot // BUCK, src_rot % BUCK
        b2, l2 = src // BUCK, src % BUCK
        counts1[c] = np.bincount(t_all * NBK + b1, minlength=T * NBK).reshape(T, NBK)
        counts2[c] = np.bincount(t_all * NBK + b2, minlength=T * NBK).reshape(T, NBK)
        pc.append((t_all, dstl, dstloc, b1, l1, b2, l2))

    K1, ops1, colbase1, coff1, CT1 = _schedule(counts1, T)
    K2, ops2, colbase2, coff2, CT2 = _schedule(counts2, T)

    arrays = []
    for c in range(W):
        t_all, dstl, dstloc, b1, l1, b2, l2 = pc[c]
        iw1, iw1d, dstl1 = _layer_arrays(l1, b1, t_all, dstl, dstloc,
                                         K1, ops1, colbase1, CT1)
        iw2, iw2d, dstl2 = _layer_arrays(l2, b2, t_all, dstl, dstloc,
                                         K2, ops2, colbase2, CT2)
        arrays.append(dict(iw1=iw1, iw1d=iw1d, dstl1=dstl1,
                           iw2=iw2, iw2d=iw2d, dstl2=dstl2))

    xT = np.zeros((F_IN, NPAD), np.float32)
    xT[:, :N] = np.asarray(x, np.float32).T
    xT = xT.astype(BF_NP)
    xT_rot = [np.roll(xT, -c * SH, axis=1) for c in range(W)]

    iota = np.broadcast_to(np.arange(P, dtype=np.float32), (P, P)).astype(BF_NP)
    ident = np.eye(P, dtype=np.float32).astype(BF_NP)

    meta = dict(N=N, E=E, T=T, SH=SH, NPAD=NPAD, BUCK=BUCK,
                ops1=tuple((g, b, tuple(cols)) for g, b, cols in ops1),
                ops2=tuple((g, b, tuple(cols)) for g, b, cols in ops2),
                coff1=tuple(coff1), coff2=tuple(coff2),
                CT1=CT1, CT2=CT2)
    return meta, arrays, xT_rot, iota, ident


# --------------------------------------------------------------------------
# device program
# --------------------------------------------------------------------------

def _build_program(meta):
    T, SH, NPAD, BUCK = meta["T"], meta["SH"], meta["NPAD"], meta["BUCK"]
    CT1, CT2 = meta["CT1"], meta["CT2"]

    nc = bacc.Bacc("TRN2", target_bir_lowering=False, debug=False, num_devices=W)

    xT_d = nc.dram_tensor("xT", [F_IN, NPAD], bf16, kind="ExternalInput")
    w1_d = nc.dram_tensor("W1", [F_IN, F_IN], f32, kind="ExternalInput")
    asrc_d = nc.dram_tensor("asrc", [1, F_IN], f32, kind="ExternalInput")
    adstv_d = nc.dram_tensor("adstv", [1, F_IN], f32, kind="ExternalInput")
    b1_d = nc.dram_tensor("b1", [1, F_IN], f32, kind="ExternalInput")
    w2_d = nc.dram_tensor("W2", [F_IN, NCLS], f32, kind="ExternalInput")
    a2s_d = nc.dram_tensor("a2s", [1, NCLS], f32, kind="ExternalInput")
    a2d_d = nc.dram_tensor("a2d", [1, NCLS], f32, kind="ExternalInput")
    b2_d = nc.dram_tensor("b2", [1, NCLS], f32, kind="ExternalInput")
    iota_d = nc.dram_tensor("iota", [P, P], bf16, kind="ExternalInput")
    ident_d = nc.dram_tensor("ident", [P, P], bf16, kind="ExternalInput")
    iw1_d = nc.dram_tensor("iw1", [P, max(8, CT1 * 8)], i16, kind="ExternalInput")
    iw1d_d = nc.dram_tensor("iw1d", [P, max(8, CT1 * 8)], i16, kind="ExternalInput")
    iw2_d = nc.dram_tensor("iw2", [P, max(8, CT2 * 8)], i16, kind="ExternalInput")
    iw2d_d = nc.dram_tensor("iw2d", [P, max(8, CT2 * 8)], i16, kind="ExternalInput")
    dstl1_d = nc.dram_tensor("dstl1", [P, max(1, CT1)], bf16, kind="ExternalInput")
    dstl2_d = nc.dram_tensor("dstl2", [P, max(1, CT2)], bf16, kind="ExternalInput")
    out_d = nc.dram_tensor("out", [SH, NCLS], f32, kind="ExternalOutput")

    T1 = nc.dram_tensor("T1", [NPAD, ES1], bf16)
    ADT = nc.dram_tensor("ADT", [SH, ES2], bf16)
    Z = nc.dram_tensor("Z", [SH, F_IN], bf16)
    T2L = nc.dram_tensor("T2L", [SH, ES2], bf16)

    with tile.TileContext(nc) as tc:
        with contextlib.ExitStack() as top:
            cpool = top.enter_context(tc.tile_pool(name="const", bufs=1))
            dram = top.enter_context(tc.tile_pool(name="dram", bufs=1, space="DRAM"))

            iota_b = cpool.tile([P, P], bf16)
            nc.sync.dma_start(out=iota_b[:], in_=iota_d[:])
            ident_b = cpool.tile([P, P], bf16)
            nc.sync.dma_start(out=ident_b[:], in_=ident_d[:])
            dstl1_b = cpool.tile([P, max(1, CT1)], bf16)
            nc.sync.dma_start(out=dstl1_b[:], in_=dstl1_d[:])
            dstl2_b = cpool.tile([P, max(1, CT2)], bf16)
            nc.sync.dma_start(out=dstl2_b[:], in_=dstl2_d[:])

            rhs1 = [cpool.tile([P, D1], bf16, name=f"rhs1_{k}") for k in range(2)]
            rhs2 = [cpool.tile([P, D2], bf16, name=f"rhs2_{k}") for k in range(2)]
            b1_b = cpool.tile([P, F_IN], f32)
            b2_b = cpool.tile([P, NCLS], f32)

            # ---- setup: broadcast rows + fold attention vectors into rhs ----
            with contextlib.ExitStack() as su:
                spool = su.enter_context(tc.tile_pool(name="setup", bufs=1))
                spsum = su.enter_context(tc.tile_pool(name="setup_ps", bufs=1, space="PSUM"))
                ones = spool.tile([1, P], f32)
                nc.vector.memset(ones[:], 1.0)

                def bcast(dram_ap, width, out_ap):
                    ps = spsum.tile([P, width], f32, tag="bps")
                    row = spool.tile([1, width], f32, tag="brow")
                    nc.sync.dma_start(out=row[:], in_=dram_ap)
                    nc.tensor.matmul(ps[:], lhsT=ones[:], rhs=row[:], start=True, stop=True)
                    nc.vector.tensor_copy(out=out_ap, in_=ps[:])

                asrc_b = spool.tile([P, F_IN], f32)
                bcast(asrc_d[:], F_IN, asrc_b[:])
                adst_b = spool.tile([P, F_IN], f32)
                bcast(adstv_d[:], F_IN, adst_b[:])
                bcast(b1_d[:], F_IN, b1_b[:])
                a2s_b = spool.tile([P, NCLS], f32)
                bcast(a2s_d[:], NCLS, a2s_b[:])
                a2d_b = spool.tile([P, NCLS], f32)
                bcast(a2d_d[:], NCLS, a2d_b[:])
                bcast(b2_d[:], NCLS, b2_b[:])

                for k in range(2):
                    w1sb = spool.tile([P, F_IN], f32, tag="w1sb")
                    nc.sync.dma_start(out=w1sb[:], in_=w1_d[k * P:(k + 1) * P, :])
                    nc.vector.tensor_copy(out=rhs1[k][:, :F_IN], in_=w1sb[:])
                    for vec_b, col in ((asrc_b, F_IN), (adst_b, F_IN + H)):
                        tmp = spool.tile([P, F_IN], f32, tag="tmp")
                        nc.vector.tensor_mul(out=tmp[:], in0=w1sb[:], in1=vec_b[:])
                        vred = spool.tile([P, H], f32, tag="vred")
                        nc.vector.tensor_reduce(
                            out=vred[:], in_=tmp[:].rearrange("p (h c) -> p h c", h=H),
                            axis=mybir.AxisListType.X, op=ADD)
                        nc.vector.tensor_copy(out=rhs1[k][:, col:col + H], in_=vred[:])

                    w2sb = spool.tile([P, NCLS], f32, tag="w2sb")
                    nc.sync.dma_start(out=w2sb[:], in_=w2_d[k * P:(k + 1) * P, :])
                    nc.vector.tensor_copy(out=rhs2[k][:, :NCLS], in_=w2sb[:])
                    for vec_b, col in ((a2s_b, NCLS), (a2d_b, NCLS + 1)):
                        tmp2 = spool.tile([P, NCLS], f32, tag="tmp2")
                        nc.vector.tensor_mul(out=tmp2[:], in0=w2sb[:], in1=vec_b[:])
                        vred2 = spool.tile([P, 1], f32, tag="vred2")
                        nc.vector.tensor_reduce(
                            out=vred2[:], in_=tmp2[:].rearrange("p (o c) -> p o c", o=1),
                            axis=mybir.AxisListType.X, op=ADD)
                        nc.vector.tensor_copy(out=rhs2[k][:, col:col + 1], in_=vred2[:])

            # ---- Phase A: T1 for all NPAD nodes (redundant per core) ----
            NB = 2048 if NPAD % 2048 == 0 else 1024
            with contextlib.ExitStack() as pa:
                apool = pa.enter_context(tc.tile_pool(name="pa", bufs=3))
                apsum = pa.enter_context(tc.tile_pool(name="pa_ps", bufs=4, space="PSUM"))
                for blk in range(NPAD // NB):
                    r0 = blk * NB
                    xt = [apool.tile([P, NB], bf16, tag=f"xt{k}", name=f"xt{k}")
                          for k in range(2)]
                    for k in range(2):
                        nc.sync.dma_start(
                            out=xt[k][:], in_=xT_d[k * P:(k + 1) * P, r0:r0 + NB])
                    hsb = apool.tile([P, NB // P, D1], bf16, tag="hsb")
                    for nt in range(NB // P):
                        ps = apsum.tile([P, D1], f32, tag="aps")
                        for k in range(2):
                            nc.tensor.matmul(
                                ps[:], lhsT=xt[k][:, nt * P:(nt + 1) * P], rhs=rhs1[k][:],
                                start=(k == 0), stop=(k == 1))
                        nc.scalar.activation(out=hsb[:, nt, :], in_=ps[:], func=Copy)
                    nc.scalar.dma_start(
                        out=T1[r0:r0 + NB, :D1].rearrange("(a p) d -> p a d", p=P),
                        in_=hsb[:])
                    if r0 < SH:
                        ncov = (min(r0 + NB, SH) - r0) // P
                        adtsb = apool.tile([P, NB // P, ES2], bf16, tag="adtsb")
                        nc.vector.tensor_copy(out=adtsb[:, :ncov, :H],
                                              in_=hsb[:, :ncov, F_IN + H:])
                        nc.scalar.dma_start(
                            out=ADT[r0:r0 + ncov * P, :].rearrange("(a p) d -> p a d", p=P),
                            in_=adtsb[:, :ncov, :])

            # ---- generic edge phase ----
            def edge_phase(table_ap, adt_ap, iw_x, iwd_x, dstl_b, sched, coff,
                           nh, hw, es_g, self_fn, flush_fn, tag):
                DG = nh * hw                       # payload width
                dself = DG + 2 * nh                # selfrow width
                # matmul order per tile -> start/stop flags
                col_seq = {}
                for oi, (g, b, cols) in enumerate(sched):
                    for j, (t, k) in enumerate(cols):
                        col_seq.setdefault(t, []).append((oi, j))
                with contextlib.ExitStack() as ep:
                    gpool = ep.enter_context(tc.tile_pool(name=f"g{tag}", bufs=2))
                    sspool = ep.enter_context(tc.tile_pool(name=f"ss{tag}", bufs=3))
                    trpool = ep.enter_context(tc.tile_pool(name=f"tr{tag}", bufs=1,
                                                           space="PSUM"))
                    adepool = ep.enter_context(tc.tile_pool(name=f"ae{tag}", bufs=1,
                                                            space="PSUM"))
                    stpool = ep.enter_context(tc.tile_pool(name=f"st{tag}", bufs=2))
                    rpool = ep.enter_context(tc.tile_pool(name=f"r{tag}", bufs=2))
                    ipool = ep.enter_context(tc.tile_pool(name=f"iw{tag}", bufs=4))
                    spool2 = ep.enter_context(tc.tile_pool(name=f"s{tag}", bufs=2))
                    selfpool = ep.enter_context(tc.tile_pool(name=f"sf{tag}",
                                                             bufs=G + 2))
                    fpool = ep.enter_context(tc.tile_pool(name=f"f{tag}", bufs=2))
                    aggp = ep.enter_context(tc.tile_pool(name=f"agg{tag}", bufs=G,
                                                         space="PSUM"))
                    agg_tiles, self_tiles = {}, {}
                    cur_group = -1
                    for oi, (g, b, cols) in enumerate(sched):
                        if g != cur_group:
                            if cur_group >= 0:
                                for t in sorted(agg_tiles):
                                    flush_fn(t, agg_tiles[t], self_tiles[t], fpool)
                                agg_tiles.clear()
                                self_tiles.clear()
                            cur_group = g
                            for t in range(g * G, min((g + 1) * G, T)):
                                sr = selfpool.tile([P, dself], bf16, tag="selfrow")
                                self_fn(sr, t)
                                self_tiles[t] = sr
                        ncol = len(cols)
                        ni = ncol * P
                        iw_t = ipool.tile([P, CMAX * 8], i16, tag="iwt")
                        nc.sync.dma_start(out=iw_t[:, :ni // 16],
                                          in_=iw_x[:, coff[oi] * 8:coff[oi] * 8 + ni // 16])
                        gt = gpool.tile([P, CMAX, es_g], bf16, tag="gt")
                        nc.gpsimd.dma_gather(
                            out_ap=gt[:, :ncol, :],
                            in_ap=table_ap[b * BUCK:(b + 1) * BUCK, :],
                            idxs_ap=iw_t[:, :ni // 16],
                            num_idxs=ni, num_idxs_reg=ni, elem_size=es_g,
                            single_packet=False)
                        st = stpool.tile([P, CMAX, P], bf16, tag="st")
                        nc.vector.tensor_tensor(
                            out=st[:, :ncol, :],
                            in0=dstl_b[:, coff[oi]:coff[oi] + ncol]
                                .unsqueeze(2).to_broadcast([P, ncol, P]),
                            in1=iota_b[:].unsqueeze(1).to_broadcast([P, ncol, P]),
                            op=ISEQ)
                        adeps = adepool.tile([P, CMAX, nh], f32, tag="adeps",
                                                 name="adeps")
                        for j, (t, k) in enumerate(cols):
                            trps = trpool.tile([P, P], bf16, tag="trps", name="trps")
                            nc.tensor.transpose(out=trps[:], in_=st[:, j, :],
                                                identity=ident_b[:])
                            ssb = sspool.tile([P, P], bf16, tag="ssb", name="ssb")
                            nc.vector.tensor_copy(out=ssb[:], in_=trps[:])
                            nc.tensor.matmul(
                                adeps[:, j, :], lhsT=ssb[:],
                                rhs=self_tiles[t][:, DG + nh:DG + 2 * nh],
                                start=True, stop=True)
                        asr = gt[:, :ncol, DG:DG + nh]
                        s = spool2.tile([P, CMAX, nh], f32, tag="s")
                        nc.vector.tensor_add(out=s[:, :ncol], in0=asr,
                                             in1=adeps[:, :ncol, :])
                        nc.vector.scalar_tensor_tensor(
                            out=s[:, :ncol], in0=s[:, :ncol], scalar=NEG,
                            in1=s[:, :ncol], op0=MULT, op1=MAX)
                        rhs = rpool.tile([P, CMAX, DG + nh], bf16, tag="rhs")
                        nc.scalar.activation(out=rhs[:, :ncol, DG:], in_=s[:, :ncol],
                                             func=Exp)
                        nc.vector.tensor_tensor(
                            out=rhs[:, :ncol, :DG].rearrange(
                                "p k (h c) -> p k h c", h=nh),
                            in0=gt[:, :ncol, :DG].rearrange(
                                "p k (h c) -> p k h c", h=nh),
                            in1=rhs[:, :ncol, DG:].rearrange(
                                "p k (h o) -> p k h o", o=1)
                                .to_broadcast([P, ncol, nh, hw]),
                            op=MULT)
                        for j, (t, k) in enumerate(cols):
                            if t not in agg_tiles:
                                agg_tiles[t] = aggp.tile([P, DG + nh], f32,
                                                         tag="agg", name="agg")
                            seq = col_seq[t]
                            nc.tensor.matmul(
                                agg_tiles[t][:], lhsT=st[:, j, :], rhs=rhs[:, j, :],
                                start=(seq[0] == (oi, j)), stop=(seq[-1] == (oi, j)))
                    for t in sorted(agg_tiles):
                        flush_fn(t, agg_tiles[t], self_tiles[t], fpool)

            # ---- Phase B: layer-1 edge phase -> Z ----
            def self1(sr, t):
                nc.sync.dma_start(out=sr[:], in_=T1[t * P:(t + 1) * P, :D1])

            def flush1(t, agg, selfrow, fpool):
                es = fpool.tile([P, H], f32, tag="es")
                nc.vector.tensor_add(out=es[:], in0=selfrow[:, F_IN:F_IN + H],
                                     in1=selfrow[:, F_IN + H:D1])
                nc.vector.scalar_tensor_tensor(
                    out=es[:], in0=es[:], scalar=NEG, in1=es[:], op0=MULT, op1=MAX)
                exs = fpool.tile([P, H], f32, tag="exs")
                nc.scalar.activation(out=exs[:], in_=es[:], func=Exp)
                selfsc = fpool.tile([P, F_IN], f32, tag="selfsc")
                nc.vector.tensor_tensor(
                    out=selfsc[:].rearrange("p (h c) -> p h c", h=H),
                    in0=selfrow[:, :F_IN].rearrange("p (h c) -> p h c", h=H),
                    in1=exs[:].rearrange("p (h o) -> p h o", o=1).to_broadcast([P, H, C]),
                    op=MULT)
                numer = fpool.tile([P, F_IN], f32, tag="numer")
                nc.vector.tensor_add(out=numer[:], in0=selfsc[:], in1=agg[:, :F_IN])
                dinv = fpool.tile([P, H], f32, tag="dinv")
                nc.vector.tensor_add(out=dinv[:], in0=exs[:], in1=agg[:, F_IN:])
                nc.vector.tensor_scalar_add(out=dinv[:], in0=dinv[:], scalar1=1e-16)
                nc.vector.reciprocal(out=dinv[:], in_=dinv[:])
                o = fpool.tile([P, F_IN], f32, tag="o")
                nc.vector.tensor_tensor(
                    out=o[:].rearrange("p (h c) -> p h c", h=H),
                    in0=numer[:].rearrange("p (h c) -> p h c", h=H),
                    in1=dinv[:].rearrange("p (h o) -> p h o", o=1).to_broadcast([P, H, C]),
                    op=MULT)
                nc.vector.tensor_add(out=o[:], in0=o[:], in1=b1_b[:])
                mmin = fpool.tile([P, F_IN], f32, tag="mmin")
                nc.vector.tensor_scalar_min(out=mmin[:], in0=o[:], scalar1=0.0)
                ex = fpool.tile([P, F_IN], f32, tag="ex")
                nc.scalar.activation(out=ex[:], in_=mmin[:], func=Exp)
                rel = fpool.tile([P, F_IN], f32, tag="rel")
                nc.vector.tensor_scalar_max(out=rel[:], in0=o[:], scalar1=0.0)
                z = fpool.tile([P, F_IN], bf16, tag="z")
                nc.vector.scalar_tensor_tensor(
                    out=z[:], in0=ex[:], scalar=-1.0, in1=rel[:], op0=ADD, op1=ADD)
                nc.sync.dma_start(out=Z[t * P:(t + 1) * P, :], in_=z[:])

            edge_phase(T1[:], ADT[:], iw1_d[:], iw1d_d[:], dstl1_b,
                       meta["ops1"], meta["coff1"], H, C, ES1,
                       self1, flush1, "b")

            # ---- Phase C: T2 = [z@W2 | a2s | a2d | pad]; AllGather ----
            T2F = dram.tile([NPAD, ES2], bf16, name="T2F", addr_space="Shared")
            with contextlib.ExitStack() as pcx:
                cpool2 = pcx.enter_context(tc.tile_pool(name="pc", bufs=3))
                cpsum = pcx.enter_context(tc.tile_pool(name="pc_ps", bufs=2, space="PSUM"))
                for t in range(T):
                    zt = [cpool2.tile([P, P], bf16, tag=f"zt{k}", name=f"zt{k}")
                          for k in range(2)]
                    for k in range(2):
                        nc.sync.dma_start(
                            out=zt[k][:], in_=Z[t * P:(t + 1) * P, k * P:(k + 1) * P],
                            transpose=True)
                    ps2 = cpsum.tile([P, D2], f32, tag="cps")
                    for k in range(2):
                        nc.tensor.matmul(ps2[:], lhsT=zt[k][:], rhs=rhs2[k][:],
                                         start=(k == 0), stop=(k == 1))
                    t2sb = cpool2.tile([P, ES2], bf16, tag="t2sb")
                    nc.scalar.activation(out=t2sb[:, :D2], in_=ps2[:], func=Copy)
                    nc.sync.dma_start(out=T2L[t * P:(t + 1) * P, :], in_=t2sb[:])

            nc.gpsimd.collective_compute(
                "AllGather", mybir.AluOpType.bypass,
                replica_groups=[list(range(W))],
                ins=[T2L[:].opt()], outs=[T2F.opt()])

            # ---- Phase D: layer-2 edge phase -> out ----
            def self2(sr, t):
                nc.sync.dma_start(out=sr[:], in_=T2L[t * P:(t + 1) * P, :D2])

            def flush2(t, agg, selfrow, fpool):
                es = fpool.tile([P, 1], f32, tag="es2")
                nc.vector.tensor_add(out=es[:], in0=selfrow[:, NCLS:NCLS + 1],
                                     in1=selfrow[:, NCLS + 1:NCLS + 2])
                nc.vector.scalar_tensor_tensor(
                    out=es[:], in0=es[:], scalar=NEG, in1=es[:], op0=MULT, op1=MAX)
                exs = fpool.tile([P, 1], f32, tag="exs2")
                nc.scalar.activation(out=exs[:], in_=es[:], func=Exp)
                selfsc = fpool.tile([P, NCLS], f32, tag="selfsc2")
                nc.vector.tensor_tensor(
                    out=selfsc[:], in0=selfrow[:, :NCLS],
                    in1=exs[:].to_broadcast([P, NCLS]), op=MULT)
                numer = fpool.tile([P, NCLS], f32, tag="numer2")
                nc.vector.tensor_add(out=numer[:], in0=selfsc[:], in1=agg[:, :NCLS])
                dinv = fpool.tile([P, 1], f32, tag="dinv2")
                nc.vector.tensor_add(out=dinv[:], in0=exs[:], in1=agg[:, NCLS:])
                nc.vector.tensor_scalar_add(out=dinv[:], in0=dinv[:], scalar1=1e-16)
                nc.vector.reciprocal(out=dinv[:], in_=dinv[:])
                o = fpool.tile([P, NCLS], f32, tag="o2")
                nc.vector.tensor_tensor(
                    out=o[:], in0=numer[:], in1=dinv[:].to_broadcast([P, NCLS]), op=MULT)
                nc.vector.tensor_add(out=o[:], in0=o[:], in1=b2_b[:])
                nc.sync.dma_start(out=out_d[t * P:(t + 1) * P, :], in_=o[:])

            edge_phase(T2F.tensor.ap(), T2L[:], iw2_d[:], iw2d_d[:], dstl2_b,
                       meta["ops2"], meta["coff2"], 1, NCLS, ES2,
                       self2, flush2, "d")

    nc.compile()
    return nc


def kernel(**inputs):
    x = np.asarray(inputs["x"], np.float32)
    edge_index = np.asarray(inputs["edge_index"])
    meta, arrays, xT_rot, iota, ident = _host_prep(x, edge_index)

    key = (meta["N"], meta["E"], meta["ops1"], meta["ops2"])
    if key not in _CACHE:
        _CACHE[key] = _build_program(meta)
    nc = _CACHE[key]

    common = {
        "W1": np.asarray(inputs["W1"], np.float32),
        "asrc": np.asarray(inputs["att_src1"], np.float32).reshape(1, -1),
        "adstv": np.asarray(inputs["att_dst1"], np.float32).reshape(1, -1),
        "b1": np.asarray(inputs["bias1"], np.float32).reshape(1, -1),
        "W2": np.asarray(inputs["W2"], np.float32),
        "a2s": np.asarray(inputs["att_src2"], np.float32).reshape(1, -1),
        "a2d": np.asarray(inputs["att_dst2"], np.float32).reshape(1, -1),
        "b2": np.asarray(inputs["bias2"], np.float32).reshape(1, -1),
        "iota": iota,
        "ident": ident,
    }
    in_maps = []
    for c in range(W):
        m = dict(common)
        m["xT"] = xT_rot[c]
        m.update(arrays[c])
        in_maps.append(m)

    res = run_bass_kernel_spmd(nc, in_maps, core_ids=list(range(W)), trace=TRACE)
    kernel.last_results = res

    N = meta["N"]
    out = np.concatenate([res.results[c]["out"] for c in range(W)], axis=0)
    return np.ascontiguousarray(out[:N])
